# revision 5
# baseline (speedup 1.0000x reference)
"""2-layer GCN (GCNConv -> ReLU -> BN -> GCNConv -> ReLU) on 8 trn2 NeuronCores.

Strategy (per core, SPMD single program):
  - Nodes are sharded two ways: P1 row-shards (original order, contiguous) for
    the x @ W1 matmul, and degree-rank round-robin dst shards for the
    aggregation phases (keeps per-tile padded degree K uniform across cores).
  - Normalization is folded into the tables: tables store h*dinv, so
    agg[d] = dinv[d] * sum_j table[src_j] with the self-loop as an extra slot.
  - BN (eval mode) is folded into W2' = diag(s) @ W2 and c2 = t @ W2 on host.
  - Gather uses indirect_dma_start with one index per partition (128 rows per
    instruction, the only HW-validated form in this environment).
  - AllGather shares each layer's table across cores between phases.

Host does only index/graph-structure preprocessing (sharding, degree counts,
padding layout, BN constant folding); all tensor math runs on device.
"""

import numpy as np

import concourse.bass as bass
import concourse.bacc as bacc
import concourse.mybir as mybir
import concourse.tile as tile
from concourse.bass_utils import run_bass_kernel_spmd

F32 = mybir.dt.float32
I32 = mybir.dt.int32

C = 8          # cores
P = 128        # partitions
H = 32         # hidden dim
D = 512        # input dim
BN_EPS = 1e-5


def _plan(n_nodes, edge_index):
    """Host-side graph preprocessing -> per-core index arrays + metadata."""
    src = np.asarray(edge_index[0], dtype=np.int64)
    dst = np.asarray(edge_index[1], dtype=np.int64)

    deg = np.bincount(dst, minlength=n_nodes).astype(np.float32) + 1.0
    dinv = (1.0 / np.sqrt(deg)).astype(np.float32)

    per = n_nodes // C                      # real rows per core (p1 + dst shards)
    SH = -(-per // 512) * 512               # shard rows, multiple of 512
    # tiles of 128 dsts; only tiles covering real rows are processed
    T_real = (per + P - 1) // P
    T_all = SH // P

    # --- dst ownership: ascending-degree rank, round-robin across cores ---
    order = np.argsort(deg, kind="stable")          # node ids by degree asc
    owner = np.empty(n_nodes, dtype=np.int64)
    pos = np.empty(n_nodes, dtype=np.int64)
    ranks = np.arange(n_nodes)
    owner[order] = ranks % C
    pos[order] = ranks // C
    assert pos.max() == per - 1

    # --- per-core CSR of in-edges by sorted dst position ---
    # edge e belongs to core owner[dst[e]] at position pos[dst[e]]
    e_owner = owner[dst]
    e_pos = pos[dst]
    # counts[c, p] = indegree of core c's p-th dst
    counts = np.zeros((C, per), dtype=np.int64)
    np.add.at(counts, (e_owner, e_pos), 1)

    # per-tile K (max indegree + 1 self-slot), uniform across cores
    K_list = []
    for t in range(T_real):
        lo, hi = t * P, min((t + 1) * P, per)
        K_list.append(int(counts[:, lo:hi].max()) + 1)
    totK = sum(K_list)
    offs = np.concatenate([[0], np.cumsum(K_list)]).astype(np.int64)

    # row mappings into the gathered tables
    # table1 rows: original-order P1 shards: node n -> (n//per)*SH + n%per
    row1 = (np.arange(n_nodes) // per) * SH + (np.arange(n_nodes) % per)
    pad1 = per  # rows [per, SH) of every core's h1 shard are zero
    # table2 rows: degree-sorted dst shards: node n -> owner*SH + pos
    row2 = owner * SH + pos
    pad2 = per  # h2 shard pad rows are explicitly zeroed

    # --- build idx arrays [C][P, totK] ---
    idx1 = np.full((C, P, totK), pad1, dtype=np.int32)
    idx2 = np.full((C, P, totK), pad2, dtype=np.int32)
    # self-loop slot first
    nodes_by_cp = np.full((C, per), -1, dtype=np.int64)
    nodes_by_cp[owner, pos] = np.arange(n_nodes)
    for c in range(C):
        for t in range(T_real):
            lo = t * P
            hi = min(lo + P, per)
            sel = nodes_by_cp[c, lo:hi]
            idx1[c, : hi - lo, offs[t]] = row1[sel]
            idx2[c, : hi - lo, offs[t]] = row2[sel]
    # remaining slots: sort edges by (owner, pos), then fill sequentially
    eorder = np.lexsort((src, e_pos, e_owner))
    so, sp, ss = e_owner[eorder], e_pos[eorder], src[eorder]
    # slot index within each (owner, pos) group
    grp = so * per + sp
    newgrp = np.ones(len(grp), dtype=bool)
    newgrp[1:] = grp[1:] != grp[:-1]
    gstart = np.where(newgrp)[0]
    slot = np.arange(len(grp)) - np.repeat(gstart, np.diff(np.concatenate([gstart, [len(grp)]])))
    tt = sp // P                                  # tile of each edge
    lane = sp % P
    col = offs[tt] + 1 + slot                     # +1 for self slot
    idx1[so, lane, col] = row1[ss]
    idx2[so, lane, col] = row2[ss]

    # --- dinv arrays ---
    dinv_p1 = np.zeros((C, P, T_all), dtype=np.float32)
    dinv_s = np.zeros((C, P, T_all), dtype=np.float32)
    for c in range(C):
        n0 = c * per
        v = dinv[n0:n0 + per]
        # p1: group g lane p <-> node n0 + g*128 + p
        full = np.zeros(SH, np.float32)
        full[:per] = v
        dinv_p1[c] = full.reshape(T_all, P).T
        fulls = np.zeros(SH, np.float32)
        fulls[:per] = dinv[nodes_by_cp[c]]
        dinv_s[c] = fulls.reshape(T_all, P).T

    meta = dict(per=per, SH=SH, T_real=T_real, T_all=T_all, K_list=K_list,
                offs=offs, totK=totK, nodes_by_cp=nodes_by_cp, dinv=dinv)
    return idx1, idx2, dinv_p1, dinv_s, meta


def _build_nc(n_nodes, meta):
    SH, T_real, T_all = meta["SH"], meta["T_real"], meta["T_all"]
    K_list, offs, totK = meta["K_list"], meta["offs"], meta["totK"]
    TAB = C * SH
    NST = T_all // 4          # 512-row supertiles in P1
    maxK = max(K_list)

    nc = bacc.Bacc("TRN2", target_bir_lowering=False, debug=False, num_devices=C)
    xT = nc.dram_tensor("xT", [D, SH], F32, kind="ExternalInput").ap()
    w1 = nc.dram_tensor("w1", [D, H], F32, kind="ExternalInput").ap()
    w2p = nc.dram_tensor("w2p", [H, H], F32, kind="ExternalInput").ap()
    b1r = nc.dram_tensor("b1r", [P, H], F32, kind="ExternalInput").ap()
    b2r = nc.dram_tensor("b2r", [P, H], F32, kind="ExternalInput").ap()
    c2r = nc.dram_tensor("c2r", [P, H], F32, kind="ExternalInput").ap()
    ident = nc.dram_tensor("ident", [P, P], F32, kind="ExternalInput").ap()
    dinvp1 = nc.dram_tensor("dinvp1", [P, T_all], F32, kind="ExternalInput").ap()
    dinvs = nc.dram_tensor("dinvs", [P, T_all], F32, kind="ExternalInput").ap()
    idx1 = nc.dram_tensor("idx1", [P, totK], I32, kind="ExternalInput").ap()
    idx2 = nc.dram_tensor("idx2", [P, totK], I32, kind="ExternalInput").ap()
    out = nc.dram_tensor("out", [SH, H], F32, kind="ExternalOutput").ap()

    with tile.TileContext(nc) as tc:
        with (
            tc.tile_pool(name="cst", bufs=1) as cst,
            tc.tile_pool(name="sb", bufs=3) as sb,
            tc.tile_pool(name="gp", bufs=3) as gp,
            tc.tile_pool(name="ps", bufs=2, space="PSUM") as ps,
            tc.tile_pool(name="dram", bufs=1, space="DRAM") as dram,
        ):
            h1s = dram.tile([SH, H], F32)
            tab1 = dram.tile([TAB, H], F32)
            h2s = dram.tile([SH, H], F32)
            tab2 = dram.tile([TAB, H], F32)

            # constants
            w1t = cst.tile([P, 4 * H], F32)
            for f in range(4):
                nc.sync.dma_start(w1t[:, f * H:(f + 1) * H],
                                  w1[f * P:(f + 1) * P, :])
            w2pt = cst.tile([H, H], F32)
            nc.sync.dma_start(w2pt[:], w2p[:, :])
            b1t = cst.tile([P, H], F32)
            nc.sync.dma_start(b1t[:], b1r[:, :])
            b2t = cst.tile([P, H], F32)
            nc.sync.dma_start(b2t[:], b2r[:, :])
            c2t = cst.tile([P, H], F32)
            nc.sync.dma_start(c2t[:], c2r[:, :])
            idt = cst.tile([P, P], F32)
            nc.sync.dma_start(idt[:], ident[:, :])
            dp1t = cst.tile([P, T_all], F32)
            nc.sync.dma_start(dp1t[:], dinvp1[:, :])
            dst_ = cst.tile([P, T_all], F32)
            nc.sync.dma_start(dst_[:], dinvs[:, :])
            ix1 = cst.tile([P, totK], I32)
            nc.sync.dma_start(ix1[:], idx1[:, :])
            ix2 = cst.tile([P, totK], I32)
            nc.sync.dma_start(ix2[:], idx2[:, :])
            zt = cst.tile([P, H], F32)
            nc.vector.memset(zt[:], 0.0)

            # ---------------- P1: h1 = (x @ W1) * dinv ----------------
            for st in range(NST):
                xt = sb.tile([P, 4 * D], F32, tag="xt")  # 4 feat chunks x 512 rows
                for f in range(4):
                    nc.sync.dma_start(
                        xt[:, f * D:(f + 1) * D],
                        xT[f * P:(f + 1) * P, st * 512:(st + 1) * 512])
                for g4 in range(4):
                    pp = ps.tile([P, H], F32, tag="p1ps")
                    for f in range(4):
                        nc.tensor.matmul(
                            pp[:],
                            lhsT=xt[:, f * D + g4 * P: f * D + (g4 + 1) * P],
                            rhs=w1t[:, f * H:(f + 1) * H],
                            start=(f == 0), stop=(f == 3))
                    ht = sb.tile([P, H], F32, tag="ht")
                    g = st * 4 + g4
                    nc.scalar.activation(ht[:], pp[:],
                                         mybir.ActivationFunctionType.Copy,
                                         scale=dp1t[:, g:g + 1])
                    nc.sync.dma_start(h1s[g * P:(g + 1) * P, :], ht[:])

            # zero pad rows of h2s (tiles >= T_real never written)
            for t in range(T_real, T_all):
                nc.sync.dma_start(h2s[t * P:(t + 1) * P, :], zt[:])

            # ---------------- AllGather 1 ----------------
            nc.gpsimd.collective_compute(
                "AllGather", mybir.AluOpType.bypass,
                replica_groups=[list(range(C))],
                ins=[h1s.opt()], outs=[tab1.opt()])

            # ---------------- Layer 1 aggregation + epilogue ----------------
            for t in range(T_real):
                K = K_list[t]
                g = gp.tile([P, maxK * H], F32, tag="g1")
                for j in range(K):
                    nc.gpsimd.indirect_dma_start(
                        out=g[:, j * H:(j + 1) * H], out_offset=None,
                        in_=tab1[:],
                        in_offset=bass.IndirectOffsetOnAxis(
                            ap=ix1[:, offs[t] + j: offs[t] + j + 1], axis=0))
                red = sb.tile([P, H], F32, tag="red")
                nc.vector.reduce_sum(
                    out=red[:],
                    in_=g[:, :K * H].rearrange("p (j f) -> p f j", f=H),
                    axis=mybir.AxisListType.X)
                nc.vector.tensor_scalar_mul(red[:], red[:], dst_[:, t:t + 1])
                nc.vector.tensor_add(red[:], red[:], b1t[:])
                nc.vector.tensor_scalar_max(red[:], red[:], 0.0)
                pt = ps.tile([H, P], F32, tag="pst")
                nc.tensor.transpose(pt[:], red[:], idt[:])
                rt = sb.tile([H, P], F32, tag="rt")
                nc.scalar.activation(rt[:], pt[:],
                                     mybir.ActivationFunctionType.Copy)
                p2 = ps.tile([P, H], F32, tag="ps2")
                nc.tensor.matmul(p2[:], lhsT=rt[:], rhs=w2pt[:],
                                 start=True, stop=True)
                h2t = sb.tile([P, H], F32, tag="h2t")
                nc.vector.tensor_add(h2t[:], p2[:], c2t[:])
                nc.vector.tensor_scalar_mul(h2t[:], h2t[:], dst_[:, t:t + 1])
                nc.sync.dma_start(h2s[t * P:(t + 1) * P, :], h2t[:])

            # ---------------- AllGather 2 ----------------
            nc.gpsimd.collective_compute(
                "AllGather", mybir.AluOpType.bypass,
                replica_groups=[list(range(C))],
                ins=[h2s.opt()], outs=[tab2.opt()])

            # ---------------- Layer 2 aggregation + epilogue ----------------
            for t in range(T_real):
                K = K_list[t]
                g = gp.tile([P, maxK * H], F32, tag="g2")
                for j in range(K):
                    nc.gpsimd.indirect_dma_start(
                        out=g[:, j * H:(j + 1) * H], out_offset=None,
                        in_=tab2[:],
                        in_offset=bass.IndirectOffsetOnAxis(
                            ap=ix2[:, offs[t] + j: offs[t] + j + 1], axis=0))
                red = sb.tile([P, H], F32, tag="red2")
                nc.vector.reduce_sum(
                    out=red[:],
                    in_=g[:, :K * H].rearrange("p (j f) -> p f j", f=H),
                    axis=mybir.AxisListType.X)
                nc.vector.tensor_scalar_mul(red[:], red[:], dst_[:, t:t + 1])
                nc.vector.tensor_add(red[:], red[:], b2t[:])
                nc.vector.tensor_scalar_max(red[:], red[:], 0.0)
                ot = sb.tile([P, H], F32, tag="ot")
                nc.vector.tensor_copy(ot[:], red[:])
                nc.sync.dma_start(out[t * P:(t + 1) * P, :], ot[:])
    nc.compile()
    return nc


def _impl(x, edge_index, W1, b1, W2, b2, gamma, beta, run_mean, run_var,
          n_nodes):
    x = np.asarray(x, np.float32)
    W1 = np.asarray(W1, np.float32)
    b1 = np.asarray(b1, np.float32)
    W2 = np.asarray(W2, np.float32)
    b2 = np.asarray(b2, np.float32)
    gamma = np.asarray(gamma, np.float32)
    beta = np.asarray(beta, np.float32)
    run_mean = np.asarray(run_mean, np.float32)
    run_var = np.asarray(run_var, np.float32)

    idx1, idx2, dinv_p1, dinv_s, meta = _plan(n_nodes, np.asarray(edge_index))
    per, SH = meta["per"], meta["SH"]

    # BN folding
    s = gamma / np.sqrt(run_var + BN_EPS)
    t = beta - run_mean * s
    W2p = (W2 * s[:, None]).astype(np.float32)
    c2 = (t @ W2).astype(np.float32)

    b1rep = np.tile(b1[None, :], (P, 1)).astype(np.float32)
    b2rep = np.tile(b2[None, :], (P, 1)).astype(np.float32)
    c2rep = np.tile(c2[None, :], (P, 1)).astype(np.float32)
    identv = np.eye(P, dtype=np.float32)

    in_maps = []
    for c in range(C):
        xs = np.zeros((SH, D), np.float32)
        xs[:per] = x[c * per:(c + 1) * per]
        xTc = np.ascontiguousarray(xs.T)
        in_maps.append({
            "xT": xTc, "w1": W1, "w2p": W2p, "b1r": b1rep, "b2r": b2rep,
            "c2r": c2rep, "ident": identv,
            "dinvp1": np.ascontiguousarray(dinv_p1[c]),
            "dinvs": np.ascontiguousarray(dinv_s[c]),
            "idx1": np.ascontiguousarray(idx1[c]),
            "idx2": np.ascontiguousarray(idx2[c]),
        })

    nc = _build_nc(n_nodes, meta)
    global _LAST_NC, _LAST_IN_MAPS
    _LAST_NC, _LAST_IN_MAPS = nc, in_maps
    res = run_bass_kernel_spmd(nc, in_maps, core_ids=list(range(C))).results

    outf = np.zeros((n_nodes, H), np.float32)
    nodes_by_cp = meta["nodes_by_cp"]
    for c in range(C):
        outf[nodes_by_cp[c]] = res[c]["out"][:per]
    return outf


def kernel(x, edge_index, W1, b1, W2, b2, gamma, beta, run_mean, run_var):
    return _impl(x, edge_index, W1, b1, W2, b2, gamma, beta, run_mean,
                 run_var, n_nodes=100000)
